# revision 63
# baseline (speedup 1.0000x reference)
"""Multi-head attention (B=16, N=1024, H=12, hd=64, DIM=768) on 8 TRN2 NeuronCores.

Sharding: data-parallel over the batch dim — each core computes 2 of the 16
batches end-to-end (qkv proj -> masked softmax attention -> out proj). No
collectives; the host scatters inputs and gathers the output.

Key tricks:
  - key packing: padded positions are masked out of the softmax anyway, so the
    host gathers only the valid key/value tokens per batch (~50% here). The
    score matmuls, exps and P.V matmuls all shrink proportionally. The packed
    key count nk is derived from the data at build time.
  - x is pre-transposed on host to xT [dim, tok] so every matmul contracts
    over the partition dim; the softmax scale is folded into the q weights.
  - scores are computed transposed, S_T[key, query]: the residual padding mask
    is a per-partition bias fused into the ScalarE exp, and exp(S_T) is
    directly the right operand layout for the P^T.V matmul.
  - softmax denominator comes free as a 65th all-ones column of V; 1/den is
    exp(-ln(den)) on ScalarE; the channel broadcast is two DVE stream
    shuffles on a shared, once-initialized tile.
  - no max-subtraction: scores are O(+-6) for this distribution, exp is safe.
  - precision: qkv-projection inputs, P, V and the out-projection run bf16
    (fp32 psum accumulation everywhere); q/k and the scores run float32r
    (full PE rate). Measured ~4e-3 absmax-relative error vs the fp32 oracle.
  - attention is software-pipelined per head (PE order S(h+1), PV(h),
    norm(h-1)), and the ScalarE-bound attention phase is back-filled with the
    rest of this batch's q/k/v projection and the previous batch's
    out-projection so the PE never idles long enough to lose its HAM boost.
"""

import numpy as np
import ml_dtypes

import concourse.bass as bass
import concourse.mybir as mybir
import concourse.tile as tile
from concourse import bacc
from concourse.bass_utils import run_bass_kernel_spmd

B, N, DIM = 16, 1024, 768
NUM_HEADS, HEAD_DIM = 12, 64
SCALE = HEAD_DIM ** -0.5
N_CORES = 8
B_LOC = B // N_CORES  # batches per core
DC = DIM // 128  # contraction chunks
F32 = mybir.dt.float32
F32R = mybir.dt.float32r
BF16 = mybir.dt.bfloat16
MASK_NEG = -30000.0


def _pin_act_table():
    """Make natural_log_exp_and_others the only table providing Exp/Ln so the
    compiler doesn't ping-pong ACT_TABLE_LOADs between exp- and ln-tables."""
    from concourse.hw_specs import get_activation_tables

    tables = get_activation_tables("gen3")
    exp = mybir.ActivationFunctionType.Exp
    ln = mybir.ActivationFunctionType.Ln
    for name, funcs in tables.items():
        if name != "natural_log_exp_and_others":
            funcs.discard(exp)
            funcs.discard(ln)


def build_bass(nk: int) -> bass.Bass:
    """nk = packed key count (multiple of 128)."""
    assert nk % 128 == 0 and 128 <= nk <= N
    kck = nk // 128

    _pin_act_table()
    nc = bacc.Bacc(trn_type="TRN2")

    xT_d = nc.dram_tensor("xT", [B_LOC, 128, DC, N], BF16, kind="ExternalInput")
    xTk_d = nc.dram_tensor("xTk", [B_LOC, 128, DC, nk], BF16, kind="ExternalInput")
    mask_d = nc.dram_tensor("mask_bias", [B_LOC, 128, kck], F32, kind="ExternalInput")
    wqkv_d = nc.dram_tensor("w_qkvT", [128, DC, 3 * DIM], BF16, kind="ExternalInput")
    wproj_d = nc.dram_tensor("w_projT", [128, DC, DIM], BF16, kind="ExternalInput")
    bproj_d = nc.dram_tensor("b_proj", [1, DIM], F32, kind="ExternalInput")
    out_d = nc.dram_tensor("out", [B_LOC, N, DIM], F32, kind="ExternalOutput")

    # key-dim chunks of <=512 that stay within one psum bank
    kchunks = [(0, min(512, nk))]
    if nk > 512:
        kchunks.append((512, nk - 512))

    from contextlib import ExitStack

    with tile.TileContext(nc) as tc, nc.allow_low_precision(
        reason="bf16 operands for full-rate PE matmuls"
    ), ExitStack() as stk:
        ep = stk.enter_context
        const = ep(tc.tile_pool(name="const", bufs=1))
        wq_pool = ep(tc.tile_pool(name="wq", bufs=2))
        x_pool = ep(tc.tile_pool(name="xp", bufs=1))
        xk_pool = ep(tc.tile_pool(name="xkp", bufs=1))
        q_pool = ep(tc.tile_pool(name="qt", bufs=1))
        k_pool = ep(tc.tile_pool(name="kt", bufs=1))
        v_pool = ep(tc.tile_pool(name="vp", bufs=1))
        pT_pool = ep(tc.tile_pool(name="pt", bufs=4))
        wv_pool = ep(tc.tile_pool(name="wv", bufs=1))
        wpj_pool = ep(tc.tile_pool(name="wpj", bufs=1))
        attn_pool = ep(tc.tile_pool(name="at", bufs=2))
        scr_pool = ep(tc.tile_pool(name="scr", bufs=3))
        mask_pool = ep(tc.tile_pool(name="msk", bufs=2))
        out_pool = ep(tc.tile_pool(name="outp", bufs=2))
        ps_pool = ep(tc.tile_pool(name="ps", bufs=3, space="PSUM"))
        ps1_pool = ep(tc.tile_pool(name="ps1", bufs=2, space="PSUM"))

        # ---- constants ----
        # shared reciprocal-broadcast tile (row 0 written by ScalarE exp,
        # rows 1-63 filled by stream shuffles; memset once so shuffle source
        # windows are never uninitialized)
        rr = const.tile([128, 1024], F32)
        nc.vector.memset(rr, 0.0)

        # broadcast b_proj to all 128 partitions via a stride-0 DMA
        bbc = const.tile([128, DIM], F32)
        bproj_bc_ap = bass.AP(
            tensor=bproj_d[0].tensor,
            offset=bproj_d[0].offset,
            ap=[[0, 128], [1, DIM]],
        )
        nc.gpsimd.dma_start(bbc, bproj_bc_ap)

        wv_sb = wv_pool.tile([128, DC, DIM], BF16, tag="wv")
        nc.sync.dma_start(wv_sb, wqkv_d[:, :, 2 * DIM : 3 * DIM])

        def alloc_batch(b):
            """Allocate batch tiles and emit input DMAs."""
            t = {}
            t["xT"] = x_pool.tile([128, DC, N], BF16, tag="xT", name="xT")
            t["xTk"] = xk_pool.tile([128, DC, nk], BF16, tag="xTk", name="xTk")
            for d in range(DC):
                nc.sync.dma_start(t["xT"][:, d, :], xT_d[b, :, d, :])
            t["mask"] = mask_pool.tile([128, kck], F32, tag="mask", name="mask")
            nc.sync.dma_start(t["mask"], mask_d[b])
            for d in range(DC):
                nc.sync.dma_start(t["xTk"][:, d, :], xTk_d[b, :, d, :])
            t["qT"] = q_pool.tile([128, DC, N], F32R, tag="qT", name="qT")
            t["kT"] = k_pool.tile([128, DC, nk], F32R, tag="kT", name="kT")
            t["vt"] = v_pool.tile(
                [128, kck, NUM_HEADS, HEAD_DIM + 1], BF16, tag="vt", name="vt"
            )
            nc.vector.memset(t["vt"][:, :, :, HEAD_DIM : HEAD_DIM + 1], 1.0)
            return t

        def make_qproj(t, f, wt_pre=None):
            def _qf():
                if wt_pre is None:
                    wt = wq_pool.tile([128, DC, 128], BF16, tag="wt", name="wt")
                    nc.sync.dma_start(wt, wqkv_d[:, :, f * 128 : (f + 1) * 128])
                else:
                    wt = wt_pre
                ps = ps_pool.tile([128, 1024], F32, tag="ps", name="psq")
                for d in range(DC):
                    for tt in range(2):
                        nc.tensor.matmul(
                            ps[:, tt * 512 : (tt + 1) * 512],
                            lhsT=wt[:, d, :],
                            rhs=t["xT"][:, d, tt * 512 : (tt + 1) * 512],
                            start=(d == 0), stop=(d == DC - 1),
                        )
                nc.vector.tensor_copy(t["qT"][:, f, :], ps[:, 0:1024])
            return _qf

        def make_kproj(t, f):
            def _kf():
                wt = wq_pool.tile([128, DC, 128], BF16, tag="wt", name="wt")
                nc.sync.dma_start(
                    wt, wqkv_d[:, :, DIM + f * 128 : DIM + (f + 1) * 128]
                )
                ps = ps_pool.tile([128, 1024], F32, tag="ps", name="psk")
                for d in range(DC):
                    for c0, cw in kchunks:
                        nc.tensor.matmul(
                            ps[:, c0 : c0 + cw],
                            lhsT=wt[:, d, :],
                            rhs=t["xTk"][:, d, c0 : c0 + cw],
                            start=(d == 0), stop=(d == DC - 1),
                        )
                nc.vector.tensor_copy(t["kT"][:, f, :], ps[:, 0:nk])
            return _kf

        def make_vproj(t, ng, t8):
            def _vp():
                ps = ps_pool.tile([128, 1024], F32, tag="ps", name="psv")
                for d in range(DC):
                    nc.tensor.matmul(
                        ps[:, 0:384],
                        lhsT=t["xTk"][:, d, t8 * 128 : (t8 + 1) * 128],
                        rhs=wv_sb[:, d, ng * 384 : (ng + 1) * 384],
                        start=(d == 0), stop=(d == DC - 1),
                    )
                nc.vector.tensor_copy(
                    t["vt"][:, t8, ng * 6 : (ng + 1) * 6, 0:HEAD_DIM],
                    ps[:, 0:384].rearrange("p (h c) -> p h c", c=HEAD_DIM),
                )
            return _vp

        def emit_scores(t, pts, h):
            half = (h % 2) * 64
            hc = h // 2
            pt = pT_pool.tile([128, kck, N], BF16, tag="pt", name="pt")
            pts[h] = pt
            for kc in range(kck):
                s = ps_pool.tile([128, 1024], F32, tag="ps", name="s")
                for tt in range(2):
                    nc.tensor.matmul(
                        s[:, tt * 512 : (tt + 1) * 512],
                        lhsT=t["kT"][half : half + 64, hc,
                                     kc * 128 : (kc + 1) * 128],
                        rhs=t["qT"][half : half + 64, hc,
                                    tt * 512 : (tt + 1) * 512],
                        start=True, stop=True,
                    )
                nc.scalar.activation(
                    out=pt[:, kc, :],
                    in_=s[:, 0:1024],
                    func=mybir.ActivationFunctionType.Exp,
                    bias=t["mask"][:, kc : kc + 1],
                    scale=1.0,
                )

        def emit_pv(t, pts, scrs, h):
            pt = pts.pop(h)
            scr = scr_pool.tile([128, 1024], F32, tag="scr", name="scr")
            scrs[h] = scr
            for tt in range(2):
                pso = ps1_pool.tile([128, 512], F32, tag="ps1", name="pso")
                for kc in range(kck):
                    nc.tensor.matmul(
                        pso[0 : HEAD_DIM + 1, :],
                        lhsT=t["vt"][:, kc, h, :],
                        rhs=pt[:, kc, tt * 512 : (tt + 1) * 512],
                        start=(kc == 0), stop=(kc == kck - 1),
                    )
                nc.vector.tensor_copy(
                    scr[0 : HEAD_DIM + 1, tt * 512 : (tt + 1) * 512],
                    pso[0 : HEAD_DIM + 1, :],
                )
            nc.scalar.activation(
                scr[64:65, :], scr[64:65, :], mybir.ActivationFunctionType.Ln
            )
            nc.scalar.activation(
                rr[0:1, :], scr[64:65, :],
                mybir.ActivationFunctionType.Exp, scale=-1.0,
            )

        def emit_norm(scrs, h, attn_dst):
            half = (h % 2) * 64
            hc = h // 2
            scr = scrs.pop(h)
            # broadcast 1/den (rr row 0) to 64 partitions with two 32-lane
            # DVE stream shuffles (mask 0 selects lane 0 of each window)
            nc.vector.stream_shuffle(rr[32:64, :], rr[0:32, :], [0] * 32)
            nc.vector.stream_shuffle(rr[0:32, :], rr[32:64, :], [0] * 32)
            nc.vector.tensor_mul(
                attn_dst[half : half + 64, hc, :],
                scr[0:64, :],
                rr[0:64, :],
            )

        def make_proj_units(b_, t8, attn_src, wp):
            state = {}

            def _unit_a():
                psp = ps_pool.tile([128, 1024], F32, tag="ps", name="psp")
                state["psp"] = psp
                for cc in range(DC):
                    nc.tensor.matmul(
                        psp[:, 0:512],
                        lhsT=attn_src[:, cc, t8 * 128 : (t8 + 1) * 128],
                        rhs=wp[:, cc, 0:512],
                        start=(cc == 0), stop=(cc == DC - 1),
                    )

            def _unit_b():
                psp = state.pop("psp")
                for cc in range(DC):
                    nc.tensor.matmul(
                        psp[:, 512:768],
                        lhsT=attn_src[:, cc, t8 * 128 : (t8 + 1) * 128],
                        rhs=wp[:, cc, 512:768],
                        start=(cc == 0), stop=(cc == DC - 1),
                    )
                ot = out_pool.tile([128, DIM], F32, tag="ot")
                nc.vector.tensor_add(ot, psp[:, 0:768], bbc)
                nc.sync.dma_start(out_d[b_, t8 * 128 : (t8 + 1) * 128, :], ot)

            return [_unit_a, _unit_b]

        # ---- main schedule ----
        pending_proj = []

        for b in range(B_LOC):
            wt0 = wq_pool.tile([128, DC, 128], BF16, tag="wt", name="wt")
            nc.sync.dma_start(wt0, wqkv_d[:, :, 0:128])
            t = alloc_batch(b)
            # serial prefix: only what head 0's scores need; the rest of the
            # early units overlap the first heads' exp work
            make_qproj(t, 0, wt_pre=wt0)()
            make_kproj(t, 0)()
            prefix_rest = [make_qproj(t, 1), make_kproj(t, 1)]
            prefix_rest += [make_vproj(t, 0, t8) for t8 in range(kck)]
            # the rest of qkv + previous batch's projection become fills
            fills = [(2 * c - 3, make_qproj(t, c)) for c in range(2, DC)]
            fills += [(2 * c - 3, make_kproj(t, c)) for c in range(2, DC)]
            fills += [(5, make_vproj(t, 1, t8)) for t8 in range(kck)]
            fills.sort(key=lambda z: z[0])
            fills += [(None, p) for p in pending_proj]
            pending_proj = []

            attn_t = attn_pool.tile([128, DC, N], BF16, tag="attn", name="attn")
            pts, scrs = {}, {}
            emit_scores(t, pts, 0)
            for u in prefix_rest:
                u()
            emit_scores(t, pts, 1)
            for h in range(NUM_HEADS):
                if h + 2 < NUM_HEADS:
                    emit_scores(t, pts, h + 2)
                if h >= 1:
                    emit_norm(scrs, h - 1, attn_t)
                emit_pv(t, pts, scrs, h)
                # fill: due items first, then spread the rest evenly
                n_emitted = 0
                while fills and fills[0][0] is not None and fills[0][0] <= h:
                    fills.pop(0)[1]()
                    n_emitted += 1
                quota = -(-len(fills) // (NUM_HEADS - h))
                while n_emitted < quota and fills:
                    fills.pop(0)[1]()
                    n_emitted += 1
            emit_norm(scrs, NUM_HEADS - 1, attn_t)
            for _, fl in fills:
                fl()

            wproj_sb = wpj_pool.tile([128, DC, DIM], BF16, tag="wpj", name="wpj")
            nc.sync.dma_start(wproj_sb, wproj_d[:, :, :])
            for t8 in range(N // 128):
                pending_proj.extend(make_proj_units(b, t8, attn_t, wproj_sb))

        for p in pending_proj:
            p()

    nc.finalize()
    return nc


def prep_inputs(x, padding_mask, w_qkv, w_proj, b_proj):
    """Host-side shard/layout/key-packing prep.

    Returns (per-core input maps, packed key count nk)."""
    x = np.asarray(x, dtype=np.float32)
    padding_mask = np.asarray(padding_mask).astype(bool)
    w_qkv = np.asarray(w_qkv, dtype=np.float32)
    w_proj = np.asarray(w_proj, dtype=np.float32)
    b_proj = np.asarray(b_proj, dtype=np.float32)

    wqkvT = np.ascontiguousarray(w_qkv.T)  # [dim, 3*dim] feature-major cols
    wqkvT[:, :DIM] *= SCALE  # fold attention scale into q features
    wqkvT_r = np.ascontiguousarray(
        wqkvT.reshape(DC, 128, 3 * DIM).transpose(1, 0, 2)
    ).astype(ml_dtypes.bfloat16)  # [128, DC, 3*dim]

    wprojT = np.ascontiguousarray(w_proj.T)  # [ch, c_out]
    wprojT_r = np.ascontiguousarray(
        wprojT.reshape(DC, 128, DIM).transpose(1, 0, 2)
    ).astype(ml_dtypes.bfloat16)  # [128, DC, dim] bf16

    bp = np.ascontiguousarray(b_proj.reshape(1, DIM))

    valid_idx = [np.nonzero(~padding_mask[b])[0] for b in range(x.shape[0])]
    nv_max = max((len(ix) for ix in valid_idx), default=1)
    nk = max(128, -(-nv_max // 128) * 128)  # round up to 128
    kck = nk // 128

    in_maps = []
    for c in range(N_CORES):
        xT_l, xTk_l, mb_l = [], [], []
        for bl in range(B_LOC):
            bg = c * B_LOC + bl
            xb = x[bg]  # [N, dim]
            xT_l.append(xb.T.reshape(DC, 128, N).transpose(1, 0, 2))
            ix = valid_idx[bg]
            xk = np.zeros((nk, DIM), dtype=np.float32)
            xk[: len(ix)] = xb[ix]
            xTk_l.append(xk.T.reshape(DC, 128, nk).transpose(1, 0, 2))
            mbias = np.full(nk, MASK_NEG, dtype=np.float32)
            mbias[: len(ix)] = 0.0
            mb_l.append(mbias.reshape(kck, 128).T)  # [128, kck]
        in_maps.append(
            {
                "xT": np.ascontiguousarray(np.stack(xT_l)).astype(
                    ml_dtypes.bfloat16
                ),
                "xTk": np.ascontiguousarray(np.stack(xTk_l)).astype(
                    ml_dtypes.bfloat16
                ),
                "mask_bias": np.ascontiguousarray(np.stack(mb_l)),
                "w_qkvT": wqkvT_r,
                "w_projT": wprojT_r,
                "b_proj": bp,
            }
        )
    return in_maps, nk


def kernel(x, padding_mask, w_qkv, w_proj, b_proj, _res_out=None):
    in_maps, nk = prep_inputs(x, padding_mask, w_qkv, w_proj, b_proj)
    nc = build_bass(nk)
    res = run_bass_kernel_spmd(nc, in_maps, core_ids=list(range(N_CORES)))
    if _res_out is not None:
        _res_out.append(res)
    out = np.concatenate([r_["out"] for r_ in res.results], axis=0)
    return out


# revision 65
# speedup vs baseline: 1.0105x; 1.0105x over previous
"""Multi-head attention (B=16, N=1024, H=12, hd=64, DIM=768) on 8 TRN2 NeuronCores.

Sharding: data-parallel over the batch dim — each core computes 2 of the 16
batches end-to-end (qkv proj -> masked softmax attention -> out proj). No
collectives; the host scatters inputs and gathers the output.

Key tricks:
  - key packing: padded positions are masked out of the softmax anyway, so the
    host gathers only the valid key/value tokens per batch (~50% here). The
    score matmuls, exps and P.V matmuls all shrink proportionally. The packed
    key count nk is derived from the data at build time.
  - x is pre-transposed on host to xT [dim, tok] so every matmul contracts
    over the partition dim; the softmax scale is folded into the q weights.
  - scores are computed transposed, S_T[key, query]: the residual padding mask
    is a per-partition bias fused into the ScalarE exp, and exp(S_T) is
    directly the right operand layout for the P^T.V matmul.
  - softmax denominator comes free as a 65th all-ones column of V; 1/den is
    exp(-ln(den)) on ScalarE; the channel broadcast is two DVE stream
    shuffles on a shared, once-initialized tile.
  - no max-subtraction: scores are O(+-6) for this distribution, exp is safe.
  - precision: qkv-projection inputs, P, V and the out-projection run bf16
    (fp32 psum accumulation everywhere); q/k and the scores run float32r
    (full PE rate). Measured ~4e-3 absmax-relative error vs the fp32 oracle.
  - attention is software-pipelined per head (PE order S(h+1), PV(h),
    norm(h-1)), and the ScalarE-bound attention phase is back-filled with the
    rest of this batch's q/k/v projection and the previous batch's
    out-projection so the PE never idles long enough to lose its HAM boost.
"""

import numpy as np
import ml_dtypes

import concourse.bass as bass
import concourse.mybir as mybir
import concourse.tile as tile
from concourse import bacc
from concourse.bass_utils import run_bass_kernel_spmd

B, N, DIM = 16, 1024, 768
NUM_HEADS, HEAD_DIM = 12, 64
SCALE = HEAD_DIM ** -0.5
N_CORES = 8
B_LOC = B // N_CORES  # batches per core
DC = DIM // 128  # contraction chunks
F32 = mybir.dt.float32
F32R = mybir.dt.float32r
BF16 = mybir.dt.bfloat16
MASK_NEG = -30000.0


def _pin_act_table():
    """Make natural_log_exp_and_others the only table providing Exp/Ln so the
    compiler doesn't ping-pong ACT_TABLE_LOADs between exp- and ln-tables."""
    from concourse.hw_specs import get_activation_tables

    tables = get_activation_tables("gen3")
    exp = mybir.ActivationFunctionType.Exp
    ln = mybir.ActivationFunctionType.Ln
    for name, funcs in tables.items():
        if name != "natural_log_exp_and_others":
            funcs.discard(exp)
            funcs.discard(ln)


def build_bass(nk: int) -> bass.Bass:
    """nk = packed key count (multiple of 128)."""
    assert nk % 128 == 0 and 128 <= nk <= N
    kck = nk // 128

    _pin_act_table()
    nc = bacc.Bacc(trn_type="TRN2")

    xT_d = nc.dram_tensor("xT", [B_LOC, 128, DC, N], BF16, kind="ExternalInput")
    xTk_d = nc.dram_tensor("xTk", [B_LOC, 128, DC, nk], BF16, kind="ExternalInput")
    mask_d = nc.dram_tensor("mask_bias", [B_LOC, 128, kck], F32, kind="ExternalInput")
    wqkv_d = nc.dram_tensor("w_qkvT", [128, DC, 3 * DIM], BF16, kind="ExternalInput")
    wproj_d = nc.dram_tensor("w_projT", [128, DC, DIM], BF16, kind="ExternalInput")
    bproj_d = nc.dram_tensor("b_proj", [1, DIM], F32, kind="ExternalInput")
    out_d = nc.dram_tensor("out", [B_LOC, N, DIM], F32, kind="ExternalOutput")

    # key-dim chunks of <=512 that stay within one psum bank
    kchunks = [(0, min(512, nk))]
    if nk > 512:
        kchunks.append((512, nk - 512))

    from contextlib import ExitStack

    with tile.TileContext(nc) as tc, nc.allow_low_precision(
        reason="bf16 operands for full-rate PE matmuls"
    ), ExitStack() as stk:
        ep = stk.enter_context
        const = ep(tc.tile_pool(name="const", bufs=1))
        wq_pool = ep(tc.tile_pool(name="wq", bufs=2))
        x_pool = ep(tc.tile_pool(name="xp", bufs=2))
        xk_pool = ep(tc.tile_pool(name="xkp", bufs=2))
        q_pool = ep(tc.tile_pool(name="qt", bufs=1))
        k_pool = ep(tc.tile_pool(name="kt", bufs=1))
        v_pool = ep(tc.tile_pool(name="vp", bufs=2))
        pT_pool = ep(tc.tile_pool(name="pt", bufs=4))
        wv_pool = ep(tc.tile_pool(name="wv", bufs=1))
        wpj_pool = ep(tc.tile_pool(name="wpj", bufs=1))
        attn_pool = ep(tc.tile_pool(name="at", bufs=2))
        scr_pool = ep(tc.tile_pool(name="scr", bufs=3))
        mask_pool = ep(tc.tile_pool(name="msk", bufs=2))
        out_pool = ep(tc.tile_pool(name="outp", bufs=2))
        ps_pool = ep(tc.tile_pool(name="ps", bufs=3, space="PSUM"))
        ps1_pool = ep(tc.tile_pool(name="ps1", bufs=2, space="PSUM"))

        # ---- constants ----
        # shared reciprocal-broadcast tile (row 0 written by ScalarE exp,
        # rows 1-63 filled by stream shuffles; memset once so shuffle source
        # windows are never uninitialized)
        rr = const.tile([128, 1024], F32)
        nc.vector.memset(rr, 0.0)

        # broadcast b_proj to all 128 partitions via a stride-0 DMA
        bbc = const.tile([128, DIM], F32)
        bproj_bc_ap = bass.AP(
            tensor=bproj_d[0].tensor,
            offset=bproj_d[0].offset,
            ap=[[0, 128], [1, DIM]],
        )
        nc.gpsimd.dma_start(bbc, bproj_bc_ap)

        wv_sb = wv_pool.tile([128, DC, DIM], BF16, tag="wv")
        nc.sync.dma_start(wv_sb, wqkv_d[:, :, 2 * DIM : 3 * DIM])

        def alloc_batch(b):
            """Allocate batch tiles and emit input DMAs."""
            t = {}
            t["xT"] = x_pool.tile([128, DC, N], BF16, tag="xT", name="xT")
            t["xTk"] = xk_pool.tile([128, DC, nk], BF16, tag="xTk", name="xTk")
            for d in range(DC):
                nc.sync.dma_start(t["xT"][:, d, :], xT_d[b, :, d, :])
            t["mask"] = mask_pool.tile([128, kck], F32, tag="mask", name="mask")
            nc.sync.dma_start(t["mask"], mask_d[b])
            for d in range(DC):
                nc.sync.dma_start(t["xTk"][:, d, :], xTk_d[b, :, d, :])
            t["qT"] = q_pool.tile([128, DC, N], F32R, tag="qT", name="qT")
            t["kT"] = k_pool.tile([128, DC, nk], F32R, tag="kT", name="kT")
            t["vt"] = v_pool.tile(
                [128, kck, NUM_HEADS, HEAD_DIM + 1], BF16, tag="vt", name="vt"
            )
            nc.vector.memset(t["vt"][:, :, :, HEAD_DIM : HEAD_DIM + 1], 1.0)
            return t

        def make_qproj(t, f, wt_pre=None):
            def _qf():
                if wt_pre is None:
                    wt = wq_pool.tile([128, DC, 128], BF16, tag="wt", name="wt")
                    nc.sync.dma_start(wt, wqkv_d[:, :, f * 128 : (f + 1) * 128])
                else:
                    wt = wt_pre
                ps = ps_pool.tile([128, 1024], F32, tag="ps", name="psq")
                for d in range(DC):
                    for tt in range(2):
                        nc.tensor.matmul(
                            ps[:, tt * 512 : (tt + 1) * 512],
                            lhsT=wt[:, d, :],
                            rhs=t["xT"][:, d, tt * 512 : (tt + 1) * 512],
                            start=(d == 0), stop=(d == DC - 1),
                        )
                nc.vector.tensor_copy(t["qT"][:, f, :], ps[:, 0:1024])
            return _qf

        def make_kproj(t, f):
            def _kf():
                wt = wq_pool.tile([128, DC, 128], BF16, tag="wt", name="wt")
                nc.sync.dma_start(
                    wt, wqkv_d[:, :, DIM + f * 128 : DIM + (f + 1) * 128]
                )
                ps = ps_pool.tile([128, 1024], F32, tag="ps", name="psk")
                for d in range(DC):
                    for c0, cw in kchunks:
                        nc.tensor.matmul(
                            ps[:, c0 : c0 + cw],
                            lhsT=wt[:, d, :],
                            rhs=t["xTk"][:, d, c0 : c0 + cw],
                            start=(d == 0), stop=(d == DC - 1),
                        )
                nc.vector.tensor_copy(t["kT"][:, f, :], ps[:, 0:nk])
            return _kf

        def make_vproj(t, ng, t8):
            def _vp():
                ps = ps_pool.tile([128, 1024], F32, tag="ps", name="psv")
                for d in range(DC):
                    nc.tensor.matmul(
                        ps[:, 0:384],
                        lhsT=t["xTk"][:, d, t8 * 128 : (t8 + 1) * 128],
                        rhs=wv_sb[:, d, ng * 384 : (ng + 1) * 384],
                        start=(d == 0), stop=(d == DC - 1),
                    )
                nc.vector.tensor_copy(
                    t["vt"][:, t8, ng * 6 : (ng + 1) * 6, 0:HEAD_DIM],
                    ps[:, 0:384].rearrange("p (h c) -> p h c", c=HEAD_DIM),
                )
            return _vp

        def emit_scores(t, pts, h):
            half = (h % 2) * 64
            hc = h // 2
            pt = pT_pool.tile([128, kck, N], BF16, tag="pt", name="pt")
            pts[h] = pt
            for kc in range(kck):
                s = ps_pool.tile([128, 1024], F32, tag="ps", name="s")
                for tt in range(2):
                    nc.tensor.matmul(
                        s[:, tt * 512 : (tt + 1) * 512],
                        lhsT=t["kT"][half : half + 64, hc,
                                     kc * 128 : (kc + 1) * 128],
                        rhs=t["qT"][half : half + 64, hc,
                                    tt * 512 : (tt + 1) * 512],
                        start=True, stop=True,
                    )
                nc.scalar.activation(
                    out=pt[:, kc, :],
                    in_=s[:, 0:1024],
                    func=mybir.ActivationFunctionType.Exp,
                    bias=t["mask"][:, kc : kc + 1],
                    scale=1.0,
                )

        def emit_pv(t, pts, scrs, h):
            pt = pts.pop(h)
            scr = scr_pool.tile([128, 1024], F32, tag="scr", name="scr")
            scrs[h] = scr
            for tt in range(2):
                pso = ps1_pool.tile([128, 512], F32, tag="ps1", name="pso")
                for kc in range(kck):
                    nc.tensor.matmul(
                        pso[0 : HEAD_DIM + 1, :],
                        lhsT=t["vt"][:, kc, h, :],
                        rhs=pt[:, kc, tt * 512 : (tt + 1) * 512],
                        start=(kc == 0), stop=(kc == kck - 1),
                    )
                nc.vector.tensor_copy(
                    scr[0 : HEAD_DIM + 1, tt * 512 : (tt + 1) * 512],
                    pso[0 : HEAD_DIM + 1, :],
                )
            nc.scalar.activation(
                scr[64:65, :], scr[64:65, :], mybir.ActivationFunctionType.Ln
            )
            nc.scalar.activation(
                rr[0:1, :], scr[64:65, :],
                mybir.ActivationFunctionType.Exp, scale=-1.0,
            )

        def emit_norm(scrs, h, attn_dst):
            half = (h % 2) * 64
            hc = h // 2
            scr = scrs.pop(h)
            # broadcast 1/den (rr row 0) to 64 partitions with two 32-lane
            # DVE stream shuffles (mask 0 selects lane 0 of each window)
            nc.vector.stream_shuffle(rr[32:64, :], rr[0:32, :], [0] * 32)
            nc.vector.stream_shuffle(rr[0:32, :], rr[32:64, :], [0] * 32)
            nc.vector.tensor_mul(
                attn_dst[half : half + 64, hc, :],
                scr[0:64, :],
                rr[0:64, :],
            )

        def make_proj_units(b_, t8, attn_src, wp):
            state = {}

            def _unit_a():
                psp = ps_pool.tile([128, 1024], F32, tag="ps", name="psp")
                state["psp"] = psp
                for cc in range(DC):
                    nc.tensor.matmul(
                        psp[:, 0:512],
                        lhsT=attn_src[:, cc, t8 * 128 : (t8 + 1) * 128],
                        rhs=wp[:, cc, 0:512],
                        start=(cc == 0), stop=(cc == DC - 1),
                    )

            def _unit_b():
                psp = state.pop("psp")
                for cc in range(DC):
                    nc.tensor.matmul(
                        psp[:, 512:768],
                        lhsT=attn_src[:, cc, t8 * 128 : (t8 + 1) * 128],
                        rhs=wp[:, cc, 512:768],
                        start=(cc == 0), stop=(cc == DC - 1),
                    )
                ot = out_pool.tile([128, DIM], F32, tag="ot")
                nc.vector.tensor_add(ot, psp[:, 0:768], bbc)
                nc.sync.dma_start(out_d[b_, t8 * 128 : (t8 + 1) * 128, :], ot)

            return [_unit_a, _unit_b]

        # ---- main schedule ----
        pending_proj = []
        t = None
        v_prefilled = False

        for b in range(B_LOC):
            if t is None:
                wt0 = wq_pool.tile([128, DC, 128], BF16, tag="wt", name="wt")
                nc.sync.dma_start(wt0, wqkv_d[:, :, 0:128])
                t = alloc_batch(b)
            else:
                wt0 = None
            # serial prefix: what heads 0-3 need (2-head score lookahead)
            make_qproj(t, 0, wt_pre=wt0)()
            make_kproj(t, 0)()
            if not v_prefilled:
                for t8 in range(kck):
                    make_vproj(t, 0, t8)()
            make_qproj(t, 1)()
            make_kproj(t, 1)()
            # the rest of qkv + previous batch's projection become fills
            fills = [(2 * c - 3, make_qproj(t, c)) for c in range(2, DC)]
            fills += [(2 * c - 3, make_kproj(t, c)) for c in range(2, DC)]
            fills += [(5, make_vproj(t, 1, t8)) for t8 in range(kck)]
            fills.sort(key=lambda z: z[0])
            fills += [(None, p) for p in pending_proj]
            pending_proj = []
            # next batch: allocate tiles now and use its v-projection (head
            # group 0) as tail fill for this batch's attention
            if b + 1 < B_LOC:
                t_next = alloc_batch(b + 1)
                fills += [(None, make_vproj(t_next, 0, t8)) for t8 in range(kck)]
                v_prefilled = True
            else:
                t_next = None

            attn_t = attn_pool.tile([128, DC, N], BF16, tag="attn", name="attn")
            pts, scrs = {}, {}
            emit_scores(t, pts, 0)
            emit_scores(t, pts, 1)
            for h in range(NUM_HEADS):
                if h + 2 < NUM_HEADS:
                    emit_scores(t, pts, h + 2)
                if h >= 1:
                    emit_norm(scrs, h - 1, attn_t)
                emit_pv(t, pts, scrs, h)
                # fill: due items first, then spread the rest evenly
                n_emitted = 0
                while fills and fills[0][0] is not None and fills[0][0] <= h:
                    fills.pop(0)[1]()
                    n_emitted += 1
                quota = -(-len(fills) // (NUM_HEADS - h))
                while n_emitted < quota and fills:
                    fills.pop(0)[1]()
                    n_emitted += 1
            emit_norm(scrs, NUM_HEADS - 1, attn_t)
            for _, fl in fills:
                fl()

            wproj_sb = wpj_pool.tile([128, DC, DIM], BF16, tag="wpj", name="wpj")
            nc.sync.dma_start(wproj_sb, wproj_d[:, :, :])
            for t8 in range(N // 128):
                pending_proj.extend(make_proj_units(b, t8, attn_t, wproj_sb))
            t = t_next

        for p in pending_proj:
            p()

    nc.finalize()
    return nc


def prep_inputs(x, padding_mask, w_qkv, w_proj, b_proj):
    """Host-side shard/layout/key-packing prep.

    Returns (per-core input maps, packed key count nk)."""
    x = np.asarray(x, dtype=np.float32)
    padding_mask = np.asarray(padding_mask).astype(bool)
    w_qkv = np.asarray(w_qkv, dtype=np.float32)
    w_proj = np.asarray(w_proj, dtype=np.float32)
    b_proj = np.asarray(b_proj, dtype=np.float32)

    wqkvT = np.ascontiguousarray(w_qkv.T)  # [dim, 3*dim] feature-major cols
    wqkvT[:, :DIM] *= SCALE  # fold attention scale into q features
    wqkvT_r = np.ascontiguousarray(
        wqkvT.reshape(DC, 128, 3 * DIM).transpose(1, 0, 2)
    ).astype(ml_dtypes.bfloat16)  # [128, DC, 3*dim]

    wprojT = np.ascontiguousarray(w_proj.T)  # [ch, c_out]
    wprojT_r = np.ascontiguousarray(
        wprojT.reshape(DC, 128, DIM).transpose(1, 0, 2)
    ).astype(ml_dtypes.bfloat16)  # [128, DC, dim] bf16

    bp = np.ascontiguousarray(b_proj.reshape(1, DIM))

    valid_idx = [np.nonzero(~padding_mask[b])[0] for b in range(x.shape[0])]
    nv_max = max((len(ix) for ix in valid_idx), default=1)
    nk = max(128, -(-nv_max // 128) * 128)  # round up to 128
    kck = nk // 128

    in_maps = []
    for c in range(N_CORES):
        xT_l, xTk_l, mb_l = [], [], []
        for bl in range(B_LOC):
            bg = c * B_LOC + bl
            xb = x[bg]  # [N, dim]
            xT_l.append(xb.T.reshape(DC, 128, N).transpose(1, 0, 2))
            ix = valid_idx[bg]
            xk = np.zeros((nk, DIM), dtype=np.float32)
            xk[: len(ix)] = xb[ix]
            xTk_l.append(xk.T.reshape(DC, 128, nk).transpose(1, 0, 2))
            mbias = np.full(nk, MASK_NEG, dtype=np.float32)
            mbias[: len(ix)] = 0.0
            mb_l.append(mbias.reshape(kck, 128).T)  # [128, kck]
        in_maps.append(
            {
                "xT": np.ascontiguousarray(np.stack(xT_l)).astype(
                    ml_dtypes.bfloat16
                ),
                "xTk": np.ascontiguousarray(np.stack(xTk_l)).astype(
                    ml_dtypes.bfloat16
                ),
                "mask_bias": np.ascontiguousarray(np.stack(mb_l)),
                "w_qkvT": wqkvT_r,
                "w_projT": wprojT_r,
                "b_proj": bp,
            }
        )
    return in_maps, nk


def kernel(x, padding_mask, w_qkv, w_proj, b_proj, _res_out=None):
    in_maps, nk = prep_inputs(x, padding_mask, w_qkv, w_proj, b_proj)
    nc = build_bass(nk)
    res = run_bass_kernel_spmd(nc, in_maps, core_ids=list(range(N_CORES)))
    if _res_out is not None:
        _res_out.append(res)
    out = np.concatenate([r_["out"] for r_ in res.results], axis=0)
    return out
